# revision 1
# baseline (speedup 1.0000x reference)
"""ChunkCrossAttention Trainium2 kernel.

Math (per reference):
  x = chunk_embeddings[0]                      # (S, L)
  k, v = split(x @ W_kv.T)                     # (S, D) each
  scores = einsum('jqd,sd->jqs', q, k) / sqrt(D), masked
  attn = softmax(scores, -1)
  out = (attn @ v) @ W_out.T + q  -> LayerNorm(gamma, beta)

Strategy (8 NeuronCores):
  - KV projection sharded over S: each core projects its own 512 keys
    (k^T, v^T in [d, s] layout straight out of the PE).
  - W_out folded into v: v' = v @ W_out.T, with two ones columns appended
    so the attention matmul also emits the softmax denominator (and the
    fp32r even-width ISA restriction is satisfied).
  - Attention partials: every core computes exp(q_all . k_loc) @ v'_loc
    over its local keys for ALL 8192 query rows (softmax without
    max-subtraction, mask folded into the Exp bias), then a ReduceScatter
    sums the partials across cores and hands each core its 1024-row
    q-shard. No K/V gather; the collective payload is tiny.
  - Two q-halves pipeline the second ReduceScatter behind the first
    half's epilogue.
  - Matmuls run as float32r (full-rate fp32 on the PE).
"""
import sys

sys.path.insert(0, "/opt/trn_rl_repo")

import numpy as np

import concourse.bacc as bacc
import concourse.mybir as mybir
import concourse.tile as tile
from concourse.bass_utils import run_bass_kernel_spmd

N_CORES = 8
J, Q, D = 64, 128, 256
S, L = 4096, 4096
S_LOC = S // N_CORES          # 512 keys per core
QR = (J // N_CORES) * Q       # 1024 query rows per core (output shard)
QALL = J * Q                  # 8192 query rows total
DP = D + 2                    # attention free dim: D outputs + denom + pad
LN_EPS = 1e-5
SCALE = 1.0 / np.sqrt(D)

F32 = mybir.dt.float32
F32R = mybir.dt.float32r
BF16 = mybir.dt.bfloat16
AF = mybir.ActivationFunctionType
ALU = mybir.AluOpType


def _r(ap):
    return ap.bitcast(F32R)


def build_program():
    nc = bacc.Bacc(None, num_devices=N_CORES)

    xT = nc.declare_dram_parameter("xT", [L, S_LOC], BF16, isOutput=False)
    wkvT = nc.declare_dram_parameter("wkvT", [L, 2 * D], BF16, isOutput=False)
    qT = nc.declare_dram_parameter("qT", [D, QALL], BF16, isOutput=False)
    qres = nc.declare_dram_parameter("qres", [QR, D], F32, isOutput=False)
    woutT = nc.declare_dram_parameter("woutT", [D, D], BF16, isOutput=False)
    maskb = nc.declare_dram_parameter("maskb", [128, S_LOC // 128], F32,
                                      isOutput=False)
    gamma = nc.declare_dram_parameter("gamma", [D], F32, isOutput=False)
    beta = nc.declare_dram_parameter("beta", [D], F32, isOutput=False)
    y = nc.declare_dram_parameter("y", [QR, D], F32, isOutput=True)

    # partial attention sums, one dram tensor per q-half:
    # slot c of rs_in{h} = partials for global q rows c*1024 + h*512 + [0,512)
    rs_in = [nc.dram_tensor(f"rs_in{h}", [N_CORES, QR // 2, DP], F32)
             for h in range(2)]
    rs_out = [nc.dram_tensor(f"rs_out{h}", [QR // 2, DP], F32)
              for h in range(2)]

    import concourse.bass as bass

    with tile.TileContext(nc) as tc:
        with tc.tile_pool(name="singles", bufs=1) as singles, \
             tc.tile_pool(name="xw", bufs=4) as xw, \
             tc.tile_pool(name="kv", bufs=1) as kvp, \
             tc.tile_pool(name="exp", bufs=4) as epool, \
             tc.tile_pool(name="part", bufs=3) as ppool, \
             tc.tile_pool(name="small", bufs=8) as small:

            # ---- phase 1: local K^T / V^T projection over the S shard ----
            ps1 = tc.tile_pool(name="ps_kv", bufs=1, space="PSUM")
            ps_kv = ps1.__enter__()
            acc = [ps_kv.tile([128, S_LOC], F32, tag=f"acc{h}", name=f"acc{h}")
                   for h in range(4)]
            n_big = L // 512
            for lb in range(n_big):
                xt = xw.tile([128, 4, S_LOC], BF16, tag="xt")
                nc.sync.dma_start(
                    out=xt,
                    in_=xT[lb * 512:(lb + 1) * 512, :].rearrange(
                        "(a p) s -> p a s", p=128))
                wt = xw.tile([128, 4, 2 * D], BF16, tag="wt")
                nc.sync.dma_start(
                    out=wt,
                    in_=wkvT[lb * 512:(lb + 1) * 512, :].rearrange(
                        "(a p) s -> p a s", p=128))
                for a in range(4):
                    first = lb == 0 and a == 0
                    last = lb == n_big - 1 and a == 3
                    for h in range(4):
                        nc.tensor.matmul(acc[h], wt[:, a, h * 128:(h + 1) * 128],
                                         xt[:, a, :], start=first, stop=last)
                if lb == 1:
                    xt_probe = xt

            # ---- constant / input loads ----
            # read-before-write probe: pins the 4MB qT load behind the lb=1
            # x-chunk so it does not steal SDMA bandwidth from the phase-1
            # stream during warmup
            qT_sb = singles.tile([128, 2, QALL], BF16)
            probe = small.tile([128, 1], BF16, tag="probe")
            nc.vector.tensor_copy(out=probe, in_=xt_probe[:, 0, 0:1])
            nc.vector.tensor_copy(out=probe, in_=qT_sb[:, 0, 0:1])
            nc.gpsimd.dma_start(out=qT_sb,
                              in_=qT.rearrange("(dc p) q -> p dc q", p=128))
            qres_sb = singles.tile([128, QR // 128, D], F32)
            nc.gpsimd.dma_start(out=qres_sb,
                              in_=qres.rearrange("(t p) d -> p t d", p=128))
            woutT_sb = singles.tile([128, 2, D], BF16)
            nc.gpsimd.dma_start(out=woutT_sb,
                              in_=woutT.rearrange("(dc p) d2 -> p dc d2", p=128))
            maskb_sb = singles.tile([128, S_LOC // 128], F32)
            nc.gpsimd.dma_start(out=maskb_sb, in_=maskb[:, :])
            g_ap = gamma[:]
            gamma_sb = singles.tile([128, D], F32)
            nc.gpsimd.dma_start(out=gamma_sb, in_=bass.AP(
                tensor=g_ap.tensor, offset=g_ap.offset,
                ap=[[0, 128], g_ap.ap[0]]))
            b_ap = beta[:]
            beta_sb = singles.tile([128, D], F32)
            nc.gpsimd.dma_start(out=beta_sb, in_=bass.AP(
                tensor=b_ap.tensor, offset=b_ap.offset,
                ap=[[0, 128], b_ap.ap[0]]))
            eps_sb = singles.tile([128, 1], F32)
            nc.vector.memset(eps_sb, LN_EPS)

            kT_loc = kvp.tile([128, 2, S_LOC], BF16)
            nc.scalar.copy(out=kT_loc[:, 0, :], in_=acc[0])
            nc.scalar.copy(out=kT_loc[:, 1, :], in_=acc[1])
            vT_loc = kvp.tile([128, 2, S_LOC], BF16)
            nc.scalar.copy(out=vT_loc[:, 0, :], in_=acc[2])
            nc.scalar.copy(out=vT_loc[:, 1, :], in_=acc[3])

            # ---- v' = v @ W_out.T, plus ones columns -> [s, DP] ----
            vp_sb = kvp.tile([128, 4, DP], BF16)
            nc.vector.memset(vp_sb, 1.0)
            for ss in range(4):
                pv = ps_kv.tile([128, D], F32, tag="pv", name="pv")
                for dc in range(2):
                    nc.tensor.matmul(
                        pv, vT_loc[:, dc, ss * 128:(ss + 1) * 128],
                        woutT_sb[:, dc, :], start=(dc == 0), stop=(dc == 1))
                nc.vector.tensor_copy(out=vp_sb[:, ss, 0:D], in_=pv)
            ps1.__exit__(None, None, None)

            # ---- phase 2: partial attention over local keys, all queries ----
            ps3 = tc.tile_pool(name="ps_at", bufs=1, space="PSUM")
            ps_at = ps3.__enter__()
            ps3b = tc.tile_pool(name="ps_sc", bufs=3, space="PSUM")
            ps_sc = ps3b.__enter__()

            n_st = S_LOC // 128                       # 4 local key tiles
            # process all even q-chunks (first RS half) then odd ones
            for qc in [0, 2, 4, 6, 8, 10, 12, 14, 1, 3, 5, 7, 9, 11, 13, 15]:
                h, c = qc % 2, qc // 2
                at = [ps_at.tile([128, DP], F32, tag=f"at{i}", name=f"at{i}")
                      for i in range(4)]
                for st in range(n_st):
                    sc = ps_sc.tile([128, 512], F32)
                    for dc in range(2):
                        nc.tensor.matmul(
                            sc, kT_loc[:, dc, st * 128:(st + 1) * 128],
                            qT_sb[:, dc, qc * 512:(qc + 1) * 512],
                            start=(dc == 0), stop=(dc == 1))
                    ex = epool.tile([128, 512], BF16)
                    nc.scalar.activation(out=ex, in_=sc, func=AF.Exp,
                                         bias=maskb_sb[:, st:st + 1], scale=SCALE)
                    for qt in range(4):
                        nc.tensor.matmul(
                            at[qt], ex[:, qt * 128:(qt + 1) * 128],
                            vp_sb[:, st, :],
                            start=(st == 0), stop=(st == n_st - 1))
                part = ppool.tile([128, 4, DP], F32, tag="part")
                for qt in range(4):
                    nc.vector.tensor_copy(out=part[:, qt, :], in_=at[qt])
                nc.sync.dma_start(
                    out=rs_in[h].rearrange("c (t p) f -> c p t f", p=128)[c],
                    in_=part)

            ps3b.__exit__(None, None, None)
            ps3.__exit__(None, None, None)

            # ---- phase 3: reduce-scatter partials, epilogue on own shard ----
            for h in range(2):
                nc.gpsimd.collective_compute(
                    "ReduceScatter", ALU.add,
                    replica_groups=[list(range(N_CORES))],
                    ins=[rs_in[h][:, :, :]], outs=[rs_out[h][:, :]])
            y_r = y.rearrange("(hh t p) d -> hh p t d", hh=2, p=128)
            for h in range(2):
                rs_sb = singles.tile([128, 4, DP], F32, name=f"rs_sb{h}")
                nc.gpsimd.dma_start(
                    out=rs_sb,
                    in_=rs_out[h].rearrange("(t p) f -> p t f", p=128))
                h_half = singles.tile([128, 4, D], F32, name=f"h_half{h}")
                for j in range(4):
                    t = 4 * h + j
                    hs = h_half[:, j, :]
                    rec = small.tile([128, 1], F32, tag="rec")
                    nc.vector.reciprocal(out=rec, in_=rs_sb[:, j, D:D + 1])
                    nc.vector.tensor_scalar_mul(out=hs, in0=rs_sb[:, j, 0:D],
                                                scalar1=rec)
                    nc.vector.tensor_add(out=hs, in0=hs, in1=qres_sb[:, t, :])
                    stats = small.tile([128, 6], F32, tag="stats")
                    nc.vector.bn_stats(out=stats, in_=hs)
                    mv = small.tile([128, 2], F32, tag="mv")
                    nc.vector.bn_aggr(out=mv, in_=stats)
                    rstd = small.tile([128, 1], F32, tag="rstd")
                    nc.scalar.activation(out=rstd, in_=mv[:, 1:2], func=AF.Sqrt,
                                         bias=eps_sb, scale=1.0)
                    nc.vector.reciprocal(out=rstd, in_=rstd)
                    nc.vector.tensor_scalar(out=hs, in0=hs,
                                            scalar1=mv[:, 0:1], scalar2=rstd,
                                            op0=ALU.subtract, op1=ALU.mult)
                    nc.vector.tensor_mul(out=hs, in0=hs, in1=gamma_sb)
                    nc.vector.tensor_add(out=hs, in0=hs, in1=beta_sb)
                nc.gpsimd.dma_start(out=y_r[h], in_=h_half)

    nc.finalize()
    return nc


_NC_CACHE = None


def _make_in_maps(inputs):
    jq = np.asarray(inputs["justice_queries"], dtype=np.float32)
    x = np.asarray(inputs["chunk_embeddings"], dtype=np.float32)[0]
    mask = np.asarray(inputs["chunk_mask"])
    wkv = np.asarray(inputs["W_kv"], dtype=np.float32)
    wout = np.asarray(inputs["W_out"], dtype=np.float32)
    gamma = np.asarray(inputs["ln_gamma"], dtype=np.float32)
    beta = np.asarray(inputs["ln_beta"], dtype=np.float32)

    import ml_dtypes
    bf16 = ml_dtypes.bfloat16
    xT = np.ascontiguousarray(x.T.astype(bf16))         # (L, S)
    wkvT = np.ascontiguousarray(wkv.T.astype(bf16))     # (L, 2D)
    flat = np.ascontiguousarray(jq.reshape(J * Q, D))   # (8192, D)
    qT = np.ascontiguousarray(flat.T.astype(bf16))      # (D, 8192)
    woutT = np.ascontiguousarray(wout.T.astype(bf16))   # (D, D)
    mb_full = np.where(mask != 0, 0.0, -1e30).astype(np.float32)

    in_maps = []
    for c in range(N_CORES):
        mb = mb_full[c * S_LOC:(c + 1) * S_LOC]
        in_maps.append({
            "xT": np.ascontiguousarray(xT[:, c * S_LOC:(c + 1) * S_LOC]),
            "wkvT": wkvT,
            "qT": qT,
            "qres": np.ascontiguousarray(flat[c * QR:(c + 1) * QR, :]),
            "woutT": woutT,
            "maskb": np.ascontiguousarray(mb.reshape(S_LOC // 128, 128).T),
            "gamma": gamma,
            "beta": beta,
        })
    return in_maps


def kernel(**inputs) -> np.ndarray:
    global _NC_CACHE
    in_maps = _make_in_maps(inputs)
    if _NC_CACHE is None:
        _NC_CACHE = build_program()
    res = run_bass_kernel_spmd(_NC_CACHE, in_maps, list(range(N_CORES)))
    out = np.concatenate([res.results[c]["y"] for c in range(N_CORES)], axis=0)
    return np.ascontiguousarray(out.reshape(J, Q, D).astype(np.float32))



# revision 3
# speedup vs baseline: 1.5779x; 1.5779x over previous
"""ChunkCrossAttention Trainium2 kernel (v2: AllGather-KV + q-sharded attention).

Math (per reference):
  x = chunk_embeddings[0]                      # (S, L)
  k, v = split(x @ W_kv.T)                     # (S, D) each
  scores = einsum('jqd,sd->jqs', q, k) / sqrt(D), masked
  attn = softmax(scores, -1)
  out = (attn @ v) @ W_out.T + q  -> LayerNorm(gamma, beta)

Strategy (8 NeuronCores):
  - KV projection sharded over S: each core projects its own 512 keys,
    in 2 chunks of 256 keys, fp8 DoubleRow matmuls (wkvT resident in
    SBUF, xT streamed).
  - W_out folded into v (v' = v @ W_out.T scaled, plus a ones column
    that makes the attention matmul emit the softmax denominator).
  - After each 256-key chunk: k^T/v' quantized to fp8 and AllGathered
    (2 chunked AGs pipeline with the remaining projection + attention).
  - Attention is query-sharded: each core computes softmax(q k^T) v'
    for its own 1024 query rows over all 4096 keys (exp without
    max-subtraction, shifted by -3 to stay inside fp8e4 range; mask
    folded into the Exp bias). Scores run as fp8 DoubleRow; the
    attn@v' matmuls accumulate over all 32 key tiles in PSUM.
  - Epilogue (denominator divide, residual, LayerNorm) per 128-row
    q tile, overlapped with the other query half.
  - fp8 scaling: W_kv is pre-scaled x64 host-side (its std 1/64 would
    land in fp8e4's subnormal range), divided back out during the
    PSUM->fp8 copies; W_out likewise x16.
"""
import sys

sys.path.insert(0, "/opt/trn_rl_repo")

import numpy as np

import concourse.bacc as bacc
import concourse.mybir as mybir
import concourse.tile as tile
from concourse.bass_utils import run_bass_kernel_spmd

N_CORES = 8
J, Q, D = 64, 128, 256
S, L = 4096, 4096
S_LOC = S // N_CORES          # 512 keys per core
QR = (J // N_CORES) * Q       # 1024 query rows per core
CH = 2                        # phase-1 key chunks per core
CK = S_LOC // CH              # 256 keys per chunk
DP = 272                      # at free dim: 256 outputs + denom + pad to x16
N_ST = S // 128               # 32 key tiles
LN_EPS = 1e-5
SCALE = 1.0 / np.sqrt(D)
EXP_SHIFT = -3.0              # folded into maskb host-side
KV_SCALE = 64.0               # host premultiplies wkvT
WO_SCALE = 16.0               # host premultiplies woutT
AT_DR = False                 # DoubleRow for attn@v' matmuls

F32 = mybir.dt.float32
FP8 = mybir.dt.float8e4
AF = mybir.ActivationFunctionType
ALU = mybir.AluOpType
DR = mybir.MatmulPerfMode.DoubleRow


def build_program():
    nc = bacc.Bacc(None, num_devices=N_CORES)

    xT = nc.declare_dram_parameter("xT", [L, S_LOC], FP8, isOutput=False)
    wkvT = nc.declare_dram_parameter("wkvT", [L, 2 * D], FP8, isOutput=False)
    qT = nc.declare_dram_parameter("qT", [D, QR], FP8, isOutput=False)
    qres = nc.declare_dram_parameter("qres", [QR, D], F32, isOutput=False)
    woutT = nc.declare_dram_parameter("woutT", [D, D], FP8, isOutput=False)
    maskb = nc.declare_dram_parameter("maskb", [128, N_ST], F32, isOutput=False)
    gamma = nc.declare_dram_parameter("gamma", [D], F32, isOutput=False)
    beta = nc.declare_dram_parameter("beta", [D], F32, isOutput=False)
    y = nc.declare_dram_parameter("y", [QR, D], F32, isOutput=True)

    # per-chunk AllGather payload: kT (2*256) + vp (2*272) fp8 per partition
    AGW = 2 * CK + 2 * DP     # 1056
    ag_in = [nc.dram_tensor(f"ag_in{c}", [128, AGW], FP8) for c in range(CH)]
    ag_out = [nc.dram_tensor(f"ag_out{c}", [N_CORES, 128, AGW], FP8,
                             addr_space="Shared") for c in range(CH)]

    import concourse.bass as bass

    with tile.TileContext(nc) as tc:
        with tc.tile_pool(name="singles", bufs=1) as singles, \
             tc.tile_pool(name="xw", bufs=3) as xw, \
             tc.tile_pool(name="kv", bufs=1) as kvp, \
             tc.tile_pool(name="exp", bufs=3) as epool, \
             tc.tile_pool(name="small", bufs=8) as small:

            # ---- resident inputs ----
            wkv_sb = singles.tile([128, L // 128, 2 * D], FP8)
            nc.sync.dma_start(
                out=wkv_sb, in_=wkvT.rearrange("(a p) e -> p a e", p=128))
            qT_sb = singles.tile([128, 2, QR], FP8)
            nc.gpsimd.dma_start(
                out=qT_sb, in_=qT.rearrange("(dc p) q -> p dc q", p=128))
            woutT_sb = singles.tile([128, 2, D], FP8)
            nc.gpsimd.dma_start(
                out=woutT_sb, in_=woutT.rearrange("(dc p) e -> p dc e", p=128))
            maskb_sb = singles.tile([128, N_ST], F32)
            nc.gpsimd.dma_start(out=maskb_sb, in_=maskb[:, :])
            qres_sb = singles.tile([128, QR // 128, D], F32)
            nc.gpsimd.dma_start(
                out=qres_sb, in_=qres.rearrange("(t p) d -> p t d", p=128))
            g_ap = gamma[:]
            gamma_sb = singles.tile([128, D], F32)
            nc.gpsimd.dma_start(out=gamma_sb, in_=bass.AP(
                tensor=g_ap.tensor, offset=g_ap.offset,
                ap=[[0, 128], g_ap.ap[0]]))
            b_ap = beta[:]
            beta_sb = singles.tile([128, D], F32)
            nc.gpsimd.dma_start(out=beta_sb, in_=bass.AP(
                tensor=b_ap.tensor, offset=b_ap.offset,
                ap=[[0, 128], b_ap.ap[0]]))
            eps_sb = singles.tile([128, 1], F32)
            nc.vector.memset(eps_sb, LN_EPS)

            kT_all = singles.tile([128, 2, S], FP8)
            vp_all = singles.tile([128, N_ST, DP], FP8)

            # ---- phase 1: project local keys, chunk by chunk; AG each ----
            ps1 = tc.tile_pool(name="ps_kv", bufs=1, space="PSUM")
            ps_kv = ps1.__enter__()
            for c in range(CH):
                acc = [ps_kv.tile([128, 512], F32, tag=f"acc{h}",
                                  name=f"acc{c}_{h}") for h in range(4)]
                for lb in range(L // 512):
                    xt = xw.tile([128, 4, CK], FP8, tag="xt")
                    nc.sync.dma_start(
                        out=xt,
                        in_=xT[lb * 512:(lb + 1) * 512,
                               c * CK:(c + 1) * CK].rearrange(
                            "(a p) s -> p a s", p=128))
                    for ap_ in range(2):
                        A = lb * 4 + 2 * ap_
                        first = lb == 0 and ap_ == 0
                        last = lb == L // 512 - 1 and ap_ == 1
                        for h in range(4):
                            nc.tensor.matmul(
                                acc[h][:, 0:CK],
                                wkv_sb[:, A:A + 2, h * 128:(h + 1) * 128],
                                xt[:, 2 * ap_:2 * ap_ + 2, :],
                                start=first, stop=last, perf_mode=DR)

                # quantize k^T, v^T to fp8 (undo the x64 W_kv prescale)
                kt_c = kvp.tile([128, 2, CK], FP8, name=f"ktc{c}")
                vt_c = kvp.tile([128, 2, CK], FP8, name=f"vtc{c}")
                for dc in range(2):
                    nc.scalar.activation(out=kt_c[:, dc, :], in_=acc[dc][:, 0:CK],
                                         func=AF.Copy, scale=1.0 / KV_SCALE)
                    nc.scalar.activation(out=vt_c[:, dc, :],
                                         in_=acc[2 + dc][:, 0:CK],
                                         func=AF.Copy, scale=1.0 / KV_SCALE)
                # v' = v @ W_out.T (scaled), plus ones column at DP col 256
                vp_c = kvp.tile([128, 2, DP], FP8, name=f"vpc{c}")
                nc.vector.memset(vp_c, 0.0)
                for ss in range(CK // 128):
                    pv = ps_kv.tile([128, 512], F32, tag="pv", name=f"pv{c}_{ss}")
                    nc.tensor.matmul(pv[:, 0:D],
                                     vt_c[:, :, ss * 128:(ss + 1) * 128],
                                     woutT_sb, start=True, stop=True,
                                     perf_mode=DR)
                    nc.scalar.activation(out=vp_c[:, ss, 0:D], in_=pv[:, 0:D],
                                         func=AF.Copy, scale=1.0 / WO_SCALE)
                nc.vector.memset(vp_c[:, :, D:D + 1], 1.0)

                # ship local chunk, gather everyone's
                nc.sync.dma_start(
                    out=ag_in[c][:, 0:2 * CK].rearrange(
                        "p (dc s) -> p dc s", dc=2),
                    in_=kt_c)
                nc.sync.dma_start(
                    out=ag_in[c][:, 2 * CK:AGW].rearrange(
                        "p (i f) -> p i f", i=2),
                    in_=vp_c)
                nc.gpsimd.collective_compute(
                    "AllGather", ALU.bypass,
                    replica_groups=[list(range(N_CORES))],
                    ins=[ag_in[c][:, :]], outs=[ag_out[c][:, :, :]])
                for r in range(N_CORES):
                    slot = c * N_CORES + r
                    nc.sync.dma_start(
                        out=kT_all[:, :, slot * CK:(slot + 1) * CK],
                        in_=ag_out[c][r, :, 0:2 * CK].rearrange(
                            "p (dc s) -> p dc s", dc=2))
                    nc.sync.dma_start(
                        out=vp_all[:, slot * 2:slot * 2 + 2, :],
                        in_=ag_out[c][r, :, 2 * CK:AGW].rearrange(
                            "p (i f) -> p i f", i=2))
            ps1.__exit__(None, None, None)

            # ---- phase 2: q-sharded attention over all keys ----
            ps2 = tc.tile_pool(name="ps_at", bufs=1, space="PSUM")
            ps_at = ps2.__enter__()
            ps3 = tc.tile_pool(name="ps_sc", bufs=3, space="PSUM")
            ps_sc = ps3.__enter__()

            y_r = y.rearrange("(hh t p) d -> hh p t d", hh=2, p=128)
            for half in range(2):
                at = [ps_at.tile([128, 512], F32, tag=f"at{qt}",
                                 name=f"at{half}_{qt}") for qt in range(4)]
                for stp in range(N_ST // 2):
                    ex2 = epool.tile([128, 2, 512], FP8, tag="ex")
                    for par in range(2):
                        st = stp * 2 + par
                        sc = ps_sc.tile([128, 512], F32, tag="sc")
                        nc.tensor.matmul(
                            sc, kT_all[:, :, st * 128:(st + 1) * 128],
                            qT_sb[:, :, half * 512:(half + 1) * 512],
                            start=True, stop=True, perf_mode=DR)
                        nc.scalar.activation(out=ex2[:, par, :], in_=sc,
                                             func=AF.Exp,
                                             bias=maskb_sb[:, st:st + 1],
                                             scale=SCALE)
                    for qt in range(4):
                        if AT_DR:
                            nc.tensor.matmul(
                                at[qt][:, 0:DP],
                                ex2[:, :, qt * 128:(qt + 1) * 128],
                                vp_all[:, stp * 2:stp * 2 + 2, :],
                                start=(stp == 0), stop=(stp == N_ST // 2 - 1),
                                perf_mode=DR)
                        else:
                            for par in range(2):
                                nc.tensor.matmul(
                                    at[qt][:, 0:DP],
                                    ex2[:, par, qt * 128:(qt + 1) * 128],
                                    vp_all[:, stp * 2 + par, :],
                                    start=(stp == 0 and par == 0),
                                    stop=(stp == N_ST // 2 - 1 and par == 1))

                # ---- epilogue: denom divide, residual, LayerNorm ----
                h_half = singles.tile([128, 4, D], F32, name=f"h_half{half}")
                for qt in range(4):
                    t = 4 * half + qt
                    hs = h_half[:, qt, :]
                    rec = small.tile([128, 1], F32, tag="rec")
                    nc.vector.reciprocal(out=rec, in_=at[qt][:, D:D + 1])
                    nc.vector.tensor_scalar_mul(out=hs, in0=at[qt][:, 0:D],
                                                scalar1=rec)
                    nc.vector.tensor_add(out=hs, in0=hs, in1=qres_sb[:, t, :])
                    stats = small.tile([128, 6], F32, tag="stats")
                    nc.vector.bn_stats(out=stats, in_=hs)
                    mv = small.tile([128, 2], F32, tag="mv")
                    nc.vector.bn_aggr(out=mv, in_=stats)
                    rstd = small.tile([128, 1], F32, tag="rstd")
                    nc.scalar.activation(out=rstd, in_=mv[:, 1:2], func=AF.Sqrt,
                                         bias=eps_sb, scale=1.0)
                    nc.vector.reciprocal(out=rstd, in_=rstd)
                    nc.vector.tensor_scalar(out=hs, in0=hs,
                                            scalar1=mv[:, 0:1], scalar2=rstd,
                                            op0=ALU.subtract, op1=ALU.mult)
                    nc.vector.tensor_mul(out=hs, in0=hs, in1=gamma_sb)
                    nc.vector.tensor_add(out=hs, in0=hs, in1=beta_sb)
                nc.gpsimd.dma_start(out=y_r[half], in_=h_half)

            ps3.__exit__(None, None, None)
            ps2.__exit__(None, None, None)

    nc.finalize()
    return nc


_NC_CACHE = None


def _make_in_maps(inputs):
    jq = np.asarray(inputs["justice_queries"], dtype=np.float32)
    x = np.asarray(inputs["chunk_embeddings"], dtype=np.float32)[0]
    mask = np.asarray(inputs["chunk_mask"])
    wkv = np.asarray(inputs["W_kv"], dtype=np.float32)
    wout = np.asarray(inputs["W_out"], dtype=np.float32)
    gamma = np.asarray(inputs["ln_gamma"], dtype=np.float32)
    beta = np.asarray(inputs["ln_beta"], dtype=np.float32)

    import ml_dtypes
    fp8 = ml_dtypes.float8_e4m3
    xT = np.ascontiguousarray(x.T.astype(fp8))                    # (L, S)
    wkvT = np.ascontiguousarray((wkv.T * KV_SCALE).astype(fp8))   # (L, 2D)
    flat = np.ascontiguousarray(jq.reshape(J * Q, D))             # (8192, D)
    qTf = flat.T.astype(fp8)                                      # (D, 8192)
    woutT = np.ascontiguousarray((wout.T * WO_SCALE).astype(fp8))  # (D, D)

    # mask bias in the AllGather key order: s = c*(8*CK) + r*CK + sloc
    # maps to original key r*S_LOC + c*CK + sloc; -3 shift keeps exp in
    # fp8e4 range (ratios cancel via the denominator).
    bias_orig = np.where(mask != 0, 0.0, -1e30).astype(np.float32) + EXP_SHIFT
    perm = np.empty(S, dtype=np.int64)
    sidx = np.arange(S)
    c_ = sidx // (N_CORES * CK)
    r_ = (sidx % (N_CORES * CK)) // CK
    sl = sidx % CK
    perm = r_ * S_LOC + c_ * CK + sl
    bias_perm = bias_orig[perm]                                   # (S,)
    mb = np.ascontiguousarray(bias_perm.reshape(N_ST, 128).T)     # (128, N_ST)

    in_maps = []
    for c in range(N_CORES):
        in_maps.append({
            "xT": np.ascontiguousarray(xT[:, c * S_LOC:(c + 1) * S_LOC]),
            "wkvT": wkvT,
            "qT": np.ascontiguousarray(qTf[:, c * QR:(c + 1) * QR]),
            "qres": np.ascontiguousarray(flat[c * QR:(c + 1) * QR, :]),
            "woutT": woutT,
            "maskb": mb,
            "gamma": gamma,
            "beta": beta,
        })
    return in_maps


def kernel(**inputs) -> np.ndarray:
    global _NC_CACHE
    in_maps = _make_in_maps(inputs)
    if _NC_CACHE is None:
        _NC_CACHE = build_program()
    res = run_bass_kernel_spmd(_NC_CACHE, in_maps, list(range(N_CORES)))
    out = np.concatenate([res.results[c]["y"] for c in range(N_CORES)], axis=0)
    return np.ascontiguousarray(out.reshape(J, Q, D).astype(np.float32))
